# revision 1
# baseline (speedup 1.0000x reference)
"""Trainium2 Bass kernel for memory-augmented causal attention.

Reference computation (b=2, n=1024, m=1024 memory, 16 heads, d_head=64):
  q = (x @ Wq) * scale ; k,v = split(x @ Wkv) ; k = [mem_k; k] ; v = [mem_v; v]
  sim = q k^T + pos_bias ; causal mask on self part ; softmax ; out = attn v
  return out @ Wo + bo

Sharding: 16 heads across 8 cores (2 heads/core), both batches on every core.
Each core computes a partial output (its heads' contribution through Wo rows);
host sums the 8 partials.

Structure (final):
  - Additive pos_bias on the PE via a bf16 identity matmul accumulated into
    the sim PSUM group.  (A multiplicative exp-bias on DVE/Pool was tried and
    is slower in practice: the full-array bias matmuls keep the PE high-
    activity clock (HAM, 2.4 GHz) engaged, and they leave DVE/Pool free for
    the normalize chains and output-projection copies.)
  - Causal i-windowing: self-attention chunks above the diagonal skip their
    fully-masked i-prefix in sims/bias/exp/AV.
  - Bias tiles are DMAed once as full 2KB rows and stay resident in SBUF;
    mem_v is host-packed so every DMA descriptor is >=1KB; ones columns are
    memset on-chip; weights are host-shuffled for contiguous row loads.
  - The first two sim chunks are emitted before the v-projection so the
    Scalar exp pipeline fills while the PE runs the v matmuls/transposes.
  - ic0's output projection is interleaved into ic1's sim stream (one half
    per two chunks); softmax normalization (recip via ones-column sums)
    overlaps on DVE/Pool; warm matmuls cover p-state across the tail.
  - PSUM: 2x [128,1024] sim buffers (4 banks) + 4 persistent AV accumulators
    (4 banks); all transient matmul outputs share the sim ring.
"""

import numpy as np

import concourse.bass as bass
import concourse.mybir as mybir
import concourse.tile as tile
from concourse import bacc
from concourse import bass_utils
from concourse.masks import make_identity

F32 = mybir.dt.float32
F32R = mybir.dt.float32r
F16 = mybir.dt.float16
BF16 = mybir.dt.bfloat16

HEADS = 16
DH = 64               # head dim
B = 2                 # batch
N = 1024              # query length
M = 1024              # memory length
JT = N + M            # total key length
DIM = 1024
SCALE = DH ** -0.5
NCORE = 8
HPC = HEADS // NCORE  # heads per core = 2

NKC = DIM // 128      # contraction chunks for projections = 8
NJ = JT // 128        # j chunks = 16
NJ_MEM = M // 128     # memory j chunks = 8
NIC = N // 512        # i chunks of 512 = 2

AV_DELAY = 2          # AV trails sims by this many j-chunks


def _self_chunks(ic):
    return (ic * 512 + 511) // 128 + 1


def _unmasked_jcs(ic):
    return list(range(NJ_MEM)) + [NJ_MEM + k for k in range(min(8, _self_chunks(ic)))]


WINDOWING = True


def _wlo(ic, jc):
    """first unmasked i-column (within the 512 i-chunk) for this j-chunk"""
    if not WINDOWING or jc < NJ_MEM:
        return 0
    return max(0, (jc - NJ_MEM) * 128 - ic * 512)


_NC_CACHE = None


def _build():
    global _NC_CACHE
    if _NC_CACHE is not None:
        return _NC_CACHE

    nc = bacc.Bacc("TRN2", target_bir_lowering=False, debug=False)

    XT = nc.dram_tensor("xT", [B, DIM, N], F16, kind="ExternalInput").ap()
    WQ = nc.dram_tensor("wq", [128, DIM], F16, kind="ExternalInput").ap()
    WK = nc.dram_tensor("wk", [128, DIM], F16, kind="ExternalInput").ap()
    WV = nc.dram_tensor("wv", [128, DIM], F16, kind="ExternalInput").ap()
    WO = nc.dram_tensor("wo", [128, DIM], F32R, kind="ExternalInput").ap()
    MKT = nc.dram_tensor("mkT", [B, 128, M], F16, kind="ExternalInput").ap()
    MV = nc.dram_tensor("mv", [B, HPC, 128, NJ_MEM * (DH + 1)], F16,
                        kind="ExternalInput").ap()
    EB = nc.dram_tensor("eb", [HPC, JT, N], BF16, kind="ExternalInput").ap()
    OUT = nc.dram_tensor("out", [B, N, DIM], F16, kind="ExternalOutput").ap()

    with tile.TileContext(nc) as tc:
        with tc.tile_pool(name="const", bufs=1) as cp, \
             tc.tile_pool(name="wts", bufs=1) as wp, \
             tc.tile_pool(name="xtp", bufs=16) as xtp, \
             tc.tile_pool(name="big", bufs=1) as bigp, \
             tc.tile_pool(name="stage", bufs=2) as stp, \
             tc.tile_pool(name="ebp", bufs=32) as ebp, \
             tc.tile_pool(name="expp", bufs=10) as expp, \
             tc.tile_pool(name="outst", bufs=4) as outstp, \
             tc.tile_pool(name="smalls", bufs=2) as smallp, \
             tc.tile_pool(name="psum", bufs=1, space="PSUM") as psp:

            # ---- input DMAs, priority order, all on the sync HW-DGE ----
            wq_t = wp.tile([128, DIM], F16)
            wk_t = wp.tile([128, DIM], F16)
            wv_t = wp.tile([128, DIM], F16)
            wo_t = wp.tile([128, DIM], F32R)
            # first kc chunk of wq lands first: matmul #1 needs only it
            nc.sync.dma_start(wq_t[:, 0:128], WQ[:, 0:128])
            nc.sync.dma_start(wq_t[:, 128:DIM], WQ[:, 128:DIM])
            nc.sync.dma_start(wk_t[:, 0:128], WK[:, 0:128])
            xts = {}
            for b in range(B):
                for kc in range(NKC):
                    xts[(b, kc)] = xtp.tile([128, N], F16,
                                            name=f"xt{b}_{kc}", tag="xt")
            for kc in range(NKC):
                nc.sync.dma_start(xts[(0, kc)][:], XT[0, kc * 128:(kc + 1) * 128, :])
            nc.sync.dma_start(wk_t[:, 128:DIM], WK[:, 128:DIM])
            for kc in range(NKC):
                nc.sync.dma_start(xts[(1, kc)][:], XT[1, kc * 128:(kc + 1) * 128, :])
            nc.sync.dma_start(wv_t[:], WV)

            qT = [bigp.tile([128, N], F16, name=f"qT{b}") for b in range(B)]
            kT = [bigp.tile([128, JT], F16, name=f"kT{b}") for b in range(B)]
            vaug = [bigp.tile([128, HPC * NJ * (DH + 1)], F16, name=f"vaug{b}")
                    for b in range(B)]
            outT = [bigp.tile([128, N], F32R, name=f"outT{b}") for b in range(B)]

            def vaug_slice(b, h, jc):
                o = (h * NJ + jc) * (DH + 1)
                return vaug[b][:, o:o + DH + 1]

            for b in range(B):
                nc.sync.dma_start(kT[b][:, 0:M], MKT[b])
            ebt = {}

            def load_eb(jcl):
                for jc in jcl:
                    for h in range(HPC):
                        t = ebp.tile([128, N], BF16, name=f"eb{h}_{jc}",
                                     tag="eb")
                        nc.sync.dma_start(
                            t[:], EB[h, jc * 128:(jc + 1) * 128, :])
                        ebt[(h, jc)] = t

            load_eb(range(0, 6))
            for b in range(B):
                for h in range(HPC):
                    nc.sync.dma_start(
                        vaug[b][:, h * NJ * (DH + 1):
                                (h * NJ + NJ_MEM) * (DH + 1)],
                        MV[b, h])
            load_eb(range(6, 16))
            nc.sync.dma_start(wo_t[:], WO)

            # ---- constants (before memsets: warms gate on identh) ----
            identh = cp.tile([128, 128], F16)
            make_identity(nc, identh[:])
            identb = cp.tile([128, 128], BF16)
            make_identity(nc, identb[:])

            # ones columns for the self chunks of vaug
            for b in range(B):
                for h in range(HPC):
                    nc.vector.memset(
                        vaug[b][:].rearrange(
                            "p (s x) -> p s x", x=DH + 1)[
                            :, h * NJ + NJ_MEM:h * NJ + NJ, DH:DH + 1],
                        1.0)

            def warm(n):
                wps = psp.tile([128, 512], F32, name="warmps", tag="simps",
                               bufs=2)
                for _ in range(n):
                    nc.tensor.matmul(wps[:, 0:128], identh[:], identh[:],
                                     start=True, stop=True,
                                     skip_group_check=True)

            def warm_simps(n):
                wps = psp.tile([128, N], F32, name="warmps2", tag="simps",
                               bufs=2)
                for _ in range(n):
                    nc.tensor.matmul(wps[:, 0:512], identh[:],
                                     qT[0][:, 0:512],
                                     start=True, stop=True,
                                     skip_group_check=True)

            copy_idx = 0

            def copy_balanced(out_ap, in_ap, eng=None):
                nonlocal copy_idx
                if eng is None:
                    eng = "v" if copy_idx % 2 == 0 else "s"
                    copy_idx += 1
                if eng == "v":
                    nc.vector.tensor_copy(out_ap, in_ap)
                else:
                    nc.scalar.copy(out_ap, in_ap)

            # =============== Phase A: q/k projections ===============
            def proj_qk(kind, b):
                wt = wq_t if kind == "q" else wk_t
                ps = psp.tile([128, N], F32, name="projps", tag="simps", bufs=2)
                # kc-outer: consume each xT tile as it lands (both i-halves
                # per tile) so the PE trails the DMA stream instead of
                # catching up and stalling
                for kc in range(NKC):
                    for icx in range(NIC):
                        nc.tensor.matmul(
                            ps[:, icx * 512:(icx + 1) * 512],
                            wt[:, kc * 128:(kc + 1) * 128],
                            xts[(b, kc)][:, icx * 512:(icx + 1) * 512],
                            start=(kc == 0), stop=(kc == NKC - 1))
                if kind == "q":
                    nc.vector.tensor_copy(qT[b][:], ps[:])
                else:
                    nc.vector.tensor_copy(kT[b][:, M:JT], ps[:])

            warm(32)
            for kind, b in (("q", 0), ("k", 0), ("q", 1), ("k", 1)):
                proj_qk(kind, b)

            # ---- v projection pieces (hooked under early ic0 chunks) ----
            vst = [stp.tile([128, N], F16, name=f"vstage{b}") for b in range(B)]
            USE_XBAR = False

            def v_proj_piece(b, icx):
                ps = psp.tile([128, 512], F32, name="vps", tag="simps",
                              bufs=2)
                for kc in range(NKC):
                    nc.tensor.matmul(
                        ps[:],
                        wv_t[:, kc * 128:(kc + 1) * 128],
                        xts[(b, kc)][:, icx * 512:(icx + 1) * 512],
                        start=(kc == 0), stop=(kc == NKC - 1))
                copy_balanced(vst[b][:, icx * 512:(icx + 1) * 512], ps[:])

            def v_transpose(b):
                if USE_XBAR:
                    # XBAR transpose DMA: vst [d, j] -> vaug [j, d] per h
                    for jb in range(8):
                        jc = NJ_MEM + jb
                        for h in range(HPC):
                            nc.sync.dma_start(
                                vaug[b][:].rearrange(
                                    "p (hh jjc x) -> p hh jjc x",
                                    hh=HPC, x=DH + 1)[:, h, jc, 0:DH],
                                vst[b][h * 64:(h + 1) * 64,
                                       jb * 128:(jb + 1) * 128],
                                transpose=True)
                    return
                for jb in range(8):
                    tp = psp.tile([128, 128], F16, name="tps",
                                  tag="simps", bufs=2)
                    nc.tensor.transpose(
                        tp[:], vst[b][:, jb * 128:(jb + 1) * 128], identh[:])
                    jc = NJ_MEM + jb
                    dst = vaug[b][:].rearrange(
                        "p (hh jjc x) -> p hh jjc x", hh=HPC, x=DH + 1)[
                        :, :, jc, 0:DH]
                    nc.vector.tensor_copy(
                        dst, tp[:].rearrange("p (hh x) -> p hh x", hh=HPC))

            # =============== attention machinery ===============
            def normalize(av, ic, b, h):
                sums_sb = smallp.tile([1, 512], F32, name="sums_sb")
                nc.vector.tensor_copy(sums_sb[:], av[h][b][DH:DH + 1, :])
                recip = smallp.tile([1, 512], F32, name="recip")
                nc.vector.reciprocal_approx_fast(recip[:], sums_sb[:])
                recipb = smallp.tile([DH, 512], F32, name="recipb")
                nc.gpsimd.partition_broadcast(recipb[:], recip[:])
                nstage = smallp.tile([DH, 512], F32R, name="nstage")
                nc.vector.tensor_tensor(
                    nstage[:], av[h][b][0:DH, :], recipb[:],
                    mybir.AluOpType.mult)
                nc.sync.dma_start(
                    outT[b][h * 64:(h + 1) * 64, ic * 512:(ic + 1) * 512],
                    nstage[:])

            def out_proj_half(b, ib):
                ob = outstp.tile([128, DIM], F16, name="ob")
                ps = psp.tile([128, N], F32, name="ops", tag="simps",
                              bufs=2)
                for dc in range(DIM // 512):
                    nc.tensor.matmul(
                        ps[:, dc * 512:(dc + 1) * 512],
                        outT[b][:, ib * 128:(ib + 1) * 128],
                        wo_t[:, dc * 512:(dc + 1) * 512],
                        start=True, stop=True)
                copy_balanced(ob[:], ps[:])
                nc.sync.dma_start(OUT[b, ib * 128:(ib + 1) * 128, :], ob[:])

            def emit_sims(ic, jc, jj):
                wlo = _wlo(ic, jc)
                simps = []
                for h in range(HPC):
                    simps.append(psp.tile([128, N], F32, name=f"simps{h}",
                                          tag="simps", bufs=2))
                for h in range(HPC):
                    for b in range(B):
                        nc.tensor.matmul(
                            simps[h][:, b * 512 + wlo:(b + 1) * 512],
                            kT[b][h * 64:(h + 1) * 64,
                                  jc * 128:(jc + 1) * 128],
                            qT[b][h * 64:(h + 1) * 64,
                                  ic * 512 + wlo:(ic + 1) * 512],
                            start=True, stop=False, skip_group_check=True)
                out = []
                for h in range(HPC):
                    for b in range(B):
                        # full-array bias add keeps the PE p-state high
                        nc.tensor.matmul(
                            simps[h][:, b * 512 + wlo:(b + 1) * 512],
                            identb[:],
                            ebt[(h, jc)][:, ic * 512 + wlo:(ic + 1) * 512],
                            start=False, stop=True, skip_group_check=True)
                    er = expp.tile([128, N], F16, name="expraw", tag="expraw")
                    if wlo == 0:
                        nc.scalar.activation(
                            er[:], simps[h][:],
                            mybir.ActivationFunctionType.Exp)
                    else:
                        nc.scalar.activation(
                            er[:].rearrange("p (b i) -> p b i",
                                            b=B)[:, :, wlo:512],
                            simps[h][:].rearrange("p (b i) -> p b i",
                                                  b=B)[:, :, wlo:512],
                            mybir.ActivationFunctionType.Exp)
                    out.append(er)
                return out

            def make_av():
                return [[psp.tile([DH + 1, 512], F32, name=f"av{hh}_{bb}",
                                  tag="avps", bufs=4) for bb in range(B)]
                        for hh in range(HPC)]

            def emit_av(av, h, jc, ic, e2, first, last):
                wlo = _wlo(ic, jc)
                for b in range(B):
                    nc.tensor.matmul(
                        av[h][b][:, wlo:512],
                        vaug_slice(b, h, jc),
                        e2[:, b * 512 + wlo:(b + 1) * 512],
                        start=first, stop=last,
                        skip_group_check=True)

            pend = []
            done_av = 0

            def flush_av(av, ic, jcs, upto):
                nonlocal done_av
                while done_av < upto:
                    jc, e2s = pend[done_av]
                    for h in range(HPC):
                        emit_av(av, h, jc, ic, e2s[h], done_av == 0,
                                done_av == len(jcs) - 1)
                    done_av += 1

            # ---- ic = 0 (first chunks fill the exp pipeline while
            #      the v projection runs on the PE) ----
            jcs0 = _unmasked_jcs(0)
            pend = []
            done_av = 0
            av0 = make_av()
            for jj, jc in enumerate(jcs0):
                pend.append((jc, emit_sims(0, jc, jj)))
                if jj == 1:
                    for b in range(B):
                        for icx in range(NIC):
                            v_proj_piece(b, icx)
                        v_transpose(b)
                if jj >= AV_DELAY:
                    flush_av(av0, 0, jcs0, jj - AV_DELAY + 1)
            flush_av(av0, 0, jcs0, len(jcs0))

            # ---- ic = 1 (ic0 normalize + out-proj interleaved) ----
            jcs1 = _unmasked_jcs(1)
            pend = []
            done_av = 0
            av1 = None
            for h in range(HPC):
                normalize(av0, 0, 0, h)
            for jj, jc in enumerate(jcs1):
                pend.append((jc, emit_sims(1, jc, jj)))
                if jj == 0:
                    av1 = make_av()
                    for h in range(HPC):
                        normalize(av0, 0, 1, h)
                if jj >= 3 and jj % 2 == 1:
                    i = (jj - 3) // 2
                    out_proj_half(i // 4, i % 4)
                if jj >= AV_DELAY:
                    flush_av(av1, 1, jcs1, jj - AV_DELAY + 1)
            flush_av(av1, 1, jcs1, len(jcs1))
            out_proj_half(1, 3)

            # ---- tail ----
            for b in range(B):
                for h in range(HPC):
                    normalize(av1, 1, b, h)
            warm_simps(24)
            for b in range(B):
                for ib in range(4, 8):
                    out_proj_half(b, ib)

    nc.compile()
    _NC_CACHE = nc
    return nc


def _prep_inputs(x, mem_k, mem_v, pos_bias, Wq, Wkv, Wo):
    """Build per-core input maps (host-side sharding)."""
    x = np.ascontiguousarray(x, dtype=np.float32)
    xT = np.ascontiguousarray(x.transpose(0, 2, 1)).astype(np.float16)

    import ml_dtypes
    pb = np.ascontiguousarray(
        pos_bias[0].transpose(0, 2, 1)).astype(np.float32)     # [16, JT, N]
    jj = np.arange(JT)[:, None]
    ii = np.arange(N)[None, :]
    mask = jj > (ii + M)
    eb = np.where(mask[None], np.float32(-1.0e9), pb).astype(ml_dtypes.bfloat16)

    def shuffle_w(w):  # [1024, 128] -> [128, 1024] with kc-major columns
        return np.ascontiguousarray(
            w.reshape(NKC, 128, 128).transpose(1, 0, 2).reshape(128, DIM))

    in_maps = []
    for c in range(NCORE):
        cs = 128 * c
        wq = shuffle_w(np.asarray(Wq[:, cs:cs + 128] * SCALE)).astype(np.float16)
        wk = shuffle_w(np.asarray(Wkv[:, cs:cs + 128])).astype(np.float16)
        wv = shuffle_w(
            np.asarray(Wkv[:, DIM + cs:DIM + cs + 128])).astype(np.float16)
        wo = np.ascontiguousarray(Wo[cs:cs + 128, :], dtype=np.float32)
        mkT = np.ascontiguousarray(
            mem_k[:, :, cs:cs + 128].transpose(0, 2, 1)).astype(np.float16)
        mv_s = mem_v[:, :, cs:cs + 128].astype(np.float16).reshape(
            B, NJ_MEM, 128, HPC, DH)
        mv = np.ones((B, HPC, 128, NJ_MEM, DH + 1), dtype=np.float16)
        for h in range(HPC):
            mv[:, h, :, :, 0:DH] = mv_s[:, :, :, h, :].transpose(0, 2, 1, 3)
        mv = mv.reshape(B, HPC, 128, NJ_MEM * (DH + 1))
        in_maps.append({
            "xT": xT,
            "wq": wq, "wk": wk, "wv": wv, "wo": wo,
            "mkT": mkT,
            "mv": np.ascontiguousarray(mv),
            "eb": np.ascontiguousarray(eb[2 * c:2 * c + 2]),
        })
    return in_maps


def kernel(x, mem_k, mem_v, pos_bias, Wq, Wkv, Wo, bo, **_kw):
    nc = _build()
    in_maps = _prep_inputs(
        np.asarray(x), np.asarray(mem_k), np.asarray(mem_v),
        np.asarray(pos_bias), np.asarray(Wq), np.asarray(Wkv), np.asarray(Wo))
    res = bass_utils.run_bass_kernel_spmd(nc, in_maps, core_ids=list(range(NCORE)))
    out = np.zeros((B, N, DIM), dtype=np.float64)
    for r in res.results:
        out += r["out"].astype(np.float64)
    out += np.asarray(bo, dtype=np.float64)[None, None, :]
    return out.astype(np.float32)



# revision 3
# speedup vs baseline: 1.0153x; 1.0153x over previous
"""Trainium2 Bass kernel for memory-augmented causal attention.

Reference computation (b=2, n=1024, m=1024 memory, 16 heads, d_head=64):
  q = (x @ Wq) * scale ; k,v = split(x @ Wkv) ; k = [mem_k; k] ; v = [mem_v; v]
  sim = q k^T + pos_bias ; causal mask on self part ; softmax ; out = attn v
  return out @ Wo + bo

Sharding: 16 heads across 8 cores (2 heads/core), both batches on every core.
Each core computes a partial output (its heads' contribution through Wo rows);
host sums the 8 partials.

Structure (final):
  - Additive pos_bias on the PE via a bf16 identity matmul accumulated into
    the sim PSUM group.  (A multiplicative exp-bias on DVE/Pool was tried and
    is slower in practice: the full-array bias matmuls keep the PE high-
    activity clock (HAM, 2.4 GHz) engaged, and they leave DVE/Pool free for
    the normalize chains and output-projection copies.)
  - Causal i-windowing: self-attention chunks above the diagonal skip their
    fully-masked i-prefix in sims/bias/exp/AV.
  - Bias tiles are DMAed once as full 2KB rows and stay resident in SBUF;
    mem_v is host-packed so every DMA descriptor is >=1KB; ones columns are
    memset on-chip; weights are host-shuffled for contiguous row loads.
  - The first two sim chunks are emitted before the v-projection so the
    Scalar exp pipeline fills while the PE runs the v matmuls/transposes.
  - ic0's output projection is interleaved into ic1's sim stream (one half
    per two chunks); softmax normalization (recip via ones-column sums)
    overlaps on DVE/Pool; warm matmuls cover p-state across the tail.
  - PSUM: 2x [128,1024] sim buffers (4 banks) + 4 persistent AV accumulators
    (4 banks); all transient matmul outputs share the sim ring.
"""

import numpy as np

import concourse.bass as bass
import concourse.mybir as mybir
import concourse.tile as tile
from concourse import bacc
from concourse import bass_utils
from concourse.masks import make_identity

F32 = mybir.dt.float32
F32R = mybir.dt.float32r
F16 = mybir.dt.float16
BF16 = mybir.dt.bfloat16
F8E4 = mybir.dt.float8e4

HEADS = 16
DH = 64               # head dim
B = 2                 # batch
N = 1024              # query length
M = 1024              # memory length
JT = N + M            # total key length
DIM = 1024
SCALE = DH ** -0.5
NCORE = 8
HPC = HEADS // NCORE  # heads per core = 2

NKC = DIM // 128      # contraction chunks for projections = 8
NJ = JT // 128        # j chunks = 16
NJ_MEM = M // 128     # memory j chunks = 8
NIC = N // 512        # i chunks of 512 = 2

AV_DELAY = 2          # AV trails sims by this many j-chunks


def _self_chunks(ic):
    return (ic * 512 + 511) // 128 + 1


def _unmasked_jcs(ic):
    return list(range(NJ_MEM)) + [NJ_MEM + k for k in range(min(8, _self_chunks(ic)))]


WINDOWING = True


def _wlo(ic, jc):
    """first unmasked i-column (within the 512 i-chunk) for this j-chunk"""
    if not WINDOWING or jc < NJ_MEM:
        return 0
    return max(0, (jc - NJ_MEM) * 128 - ic * 512)


_NC_CACHE = None


def _build():
    global _NC_CACHE
    if _NC_CACHE is not None:
        return _NC_CACHE

    nc = bacc.Bacc("TRN2", target_bir_lowering=False, debug=False)

    XT = nc.dram_tensor("xT", [B, DIM, N], F16, kind="ExternalInput").ap()
    WQ = nc.dram_tensor("wq", [128, DIM], F16, kind="ExternalInput").ap()
    WK = nc.dram_tensor("wk", [128, DIM], F16, kind="ExternalInput").ap()
    WV = nc.dram_tensor("wv", [128, DIM], F16, kind="ExternalInput").ap()
    WO = nc.dram_tensor("wo", [128, DIM], F16, kind="ExternalInput").ap()
    MKT = nc.dram_tensor("mkT", [B, 128, M], F16, kind="ExternalInput").ap()
    MV = nc.dram_tensor("mv", [B, HPC, 128, NJ_MEM * (DH + 1)], F16,
                        kind="ExternalInput").ap()
    EB = nc.dram_tensor("eb", [HPC, JT, N], F8E4, kind="ExternalInput").ap()
    OUT = nc.dram_tensor("out", [B, N, DIM], F16, kind="ExternalOutput").ap()

    with tile.TileContext(nc) as tc:
        with tc.tile_pool(name="const", bufs=1) as cp, \
             tc.tile_pool(name="wts", bufs=1) as wp, \
             tc.tile_pool(name="xtp", bufs=16) as xtp, \
             tc.tile_pool(name="big", bufs=1) as bigp, \
             tc.tile_pool(name="stage", bufs=2) as stp, \
             tc.tile_pool(name="ebp", bufs=32) as ebp, \
             tc.tile_pool(name="expp", bufs=10) as expp, \
             tc.tile_pool(name="outst", bufs=4) as outstp, \
             tc.tile_pool(name="smalls", bufs=2) as smallp, \
             tc.tile_pool(name="psum", bufs=1, space="PSUM") as psp:

            # ---- input DMAs, priority order, all on the sync HW-DGE ----
            wq_t = wp.tile([128, DIM], F16)
            wk_t = wp.tile([128, DIM], F16)
            wv_t = wp.tile([128, DIM], F16)
            wo_t = wp.tile([128, DIM], F16)
            # first kc chunk of wq lands first: matmul #1 needs only it
            nc.sync.dma_start(wq_t[:, 0:128], WQ[:, 0:128])
            nc.sync.dma_start(wq_t[:, 128:DIM], WQ[:, 128:DIM])
            nc.sync.dma_start(wk_t[:, 0:128], WK[:, 0:128])
            xts = {}
            for b in range(B):
                for kc in range(NKC):
                    xts[(b, kc)] = xtp.tile([128, N], F16,
                                            name=f"xt{b}_{kc}", tag="xt")
            for kc in range(NKC):
                nc.sync.dma_start(xts[(0, kc)][:], XT[0, kc * 128:(kc + 1) * 128, :])
            nc.sync.dma_start(wk_t[:, 128:DIM], WK[:, 128:DIM])
            for kc in range(NKC):
                nc.sync.dma_start(xts[(1, kc)][:], XT[1, kc * 128:(kc + 1) * 128, :])
            nc.sync.dma_start(wv_t[:], WV)

            qT = [bigp.tile([128, N], F16, name=f"qT{b}") for b in range(B)]
            kT = [bigp.tile([128, JT], F16, name=f"kT{b}") for b in range(B)]
            vaug = [bigp.tile([128, HPC * NJ * (DH + 1)], F16, name=f"vaug{b}")
                    for b in range(B)]
            outT = [bigp.tile([128, N], F16, name=f"outT{b}") for b in range(B)]

            def vaug_slice(b, h, jc):
                o = (h * NJ + jc) * (DH + 1)
                return vaug[b][:, o:o + DH + 1]

            for b in range(B):
                nc.sync.dma_start(kT[b][:, 0:M], MKT[b])
            ebt = {}

            def load_eb(jcl):
                for jc in jcl:
                    for h in range(HPC):
                        t = ebp.tile([128, N], F8E4, name=f"eb{h}_{jc}",
                                     tag="eb")
                        nc.sync.dma_start(
                            t[:], EB[h, jc * 128:(jc + 1) * 128, :])
                        ebt[(h, jc)] = t

            load_eb(range(0, 6))
            for b in range(B):
                for h in range(HPC):
                    nc.sync.dma_start(
                        vaug[b][:, h * NJ * (DH + 1):
                                (h * NJ + NJ_MEM) * (DH + 1)],
                        MV[b, h])
            load_eb(range(6, 16))
            nc.sync.dma_start(wo_t[:], WO)

            # ---- constants (before memsets: warms gate on identh) ----
            identh = cp.tile([128, 128], F16)
            make_identity(nc, identh[:])
            identb = cp.tile([128, 128], F8E4)
            make_identity(nc, identb[:])

            # ones columns for the self chunks of vaug
            for b in range(B):
                for h in range(HPC):
                    nc.vector.memset(
                        vaug[b][:].rearrange(
                            "p (s x) -> p s x", x=DH + 1)[
                            :, h * NJ + NJ_MEM:h * NJ + NJ, DH:DH + 1],
                        1.0)

            def warm(n):
                wps = psp.tile([128, 512], F32, name="warmps", tag="simps",
                               bufs=2)
                for _ in range(n):
                    nc.tensor.matmul(wps[:, 0:128], identh[:], identh[:],
                                     start=True, stop=True,
                                     skip_group_check=True)

            def warm_simps(n):
                wps = psp.tile([128, N], F32, name="warmps2", tag="simps",
                               bufs=2)
                for _ in range(n):
                    nc.tensor.matmul(wps[:, 0:512], identh[:],
                                     qT[0][:, 0:512],
                                     start=True, stop=True,
                                     skip_group_check=True)

            copy_idx = 0

            def copy_balanced(out_ap, in_ap, eng=None):
                nonlocal copy_idx
                if eng is None:
                    eng = "v" if copy_idx % 2 == 0 else "s"
                    copy_idx += 1
                if eng == "v":
                    nc.vector.tensor_copy(out_ap, in_ap)
                else:
                    nc.scalar.copy(out_ap, in_ap)

            # =============== Phase A: q/k projections ===============
            def proj_qk(kind, b):
                wt = wq_t if kind == "q" else wk_t
                ps = psp.tile([128, N], F32, name="projps", tag="simps", bufs=2)
                # kc-outer: consume each xT tile as it lands (both i-halves
                # per tile) so the PE trails the DMA stream instead of
                # catching up and stalling
                for kc in range(NKC):
                    for icx in range(NIC):
                        nc.tensor.matmul(
                            ps[:, icx * 512:(icx + 1) * 512],
                            wt[:, kc * 128:(kc + 1) * 128],
                            xts[(b, kc)][:, icx * 512:(icx + 1) * 512],
                            start=(kc == 0), stop=(kc == NKC - 1))
                if kind == "q":
                    nc.vector.tensor_copy(qT[b][:], ps[:])
                else:
                    nc.vector.tensor_copy(kT[b][:, M:JT], ps[:])

            warm(32)
            for kind, b in (("q", 0), ("k", 0), ("q", 1), ("k", 1)):
                proj_qk(kind, b)

            # ---- v projection pieces (hooked under early ic0 chunks) ----
            vst = [stp.tile([128, N], F16, name=f"vstage{b}") for b in range(B)]
            USE_XBAR = False

            def v_proj_piece(b, icx):
                ps = psp.tile([128, 512], F32, name="vps", tag="simps",
                              bufs=2)
                for kc in range(NKC):
                    nc.tensor.matmul(
                        ps[:],
                        wv_t[:, kc * 128:(kc + 1) * 128],
                        xts[(b, kc)][:, icx * 512:(icx + 1) * 512],
                        start=(kc == 0), stop=(kc == NKC - 1))
                copy_balanced(vst[b][:, icx * 512:(icx + 1) * 512], ps[:])

            def v_transpose(b):
                if USE_XBAR:
                    # XBAR transpose DMA: vst [d, j] -> vaug [j, d] per h
                    for jb in range(8):
                        jc = NJ_MEM + jb
                        for h in range(HPC):
                            nc.sync.dma_start(
                                vaug[b][:].rearrange(
                                    "p (hh jjc x) -> p hh jjc x",
                                    hh=HPC, x=DH + 1)[:, h, jc, 0:DH],
                                vst[b][h * 64:(h + 1) * 64,
                                       jb * 128:(jb + 1) * 128],
                                transpose=True)
                    return
                for jb in range(8):
                    tp = psp.tile([128, 128], F16, name="tps",
                                  tag="simps", bufs=2)
                    nc.tensor.transpose(
                        tp[:], vst[b][:, jb * 128:(jb + 1) * 128], identh[:])
                    jc = NJ_MEM + jb
                    dst = vaug[b][:].rearrange(
                        "p (hh jjc x) -> p hh jjc x", hh=HPC, x=DH + 1)[
                        :, :, jc, 0:DH]
                    nc.vector.tensor_copy(
                        dst, tp[:].rearrange("p (hh x) -> p hh x", hh=HPC))

            # =============== attention machinery ===============
            def normalize(av, ic, b, h):
                sums_sb = smallp.tile([1, 512], F32, name="sums_sb")
                nc.vector.tensor_copy(sums_sb[:], av[h][b][DH:DH + 1, :])
                recip = smallp.tile([1, 512], F32, name="recip")
                nc.vector.reciprocal_approx_fast(recip[:], sums_sb[:])
                recipb = smallp.tile([DH, 512], F32, name="recipb")
                nc.gpsimd.partition_broadcast(recipb[:], recip[:])
                nc.vector.tensor_tensor(
                    outT[b][h * 64:(h + 1) * 64, ic * 512:(ic + 1) * 512],
                    av[h][b][0:DH, :], recipb[:],
                    mybir.AluOpType.mult)

            def out_proj_half(b, ib):
                ob = outstp.tile([128, DIM], F16, name="ob")
                ps = psp.tile([128, N], F32, name="ops", tag="simps",
                              bufs=2)
                for dc in range(DIM // 512):
                    nc.tensor.matmul(
                        ps[:, dc * 512:(dc + 1) * 512],
                        outT[b][:, ib * 128:(ib + 1) * 128],
                        wo_t[:, dc * 512:(dc + 1) * 512],
                        start=True, stop=True)
                copy_balanced(ob[:], ps[:])
                nc.sync.dma_start(OUT[b, ib * 128:(ib + 1) * 128, :], ob[:])

            def emit_sims(ic, jc, jj):
                wlo = _wlo(ic, jc)
                simps = []
                for h in range(HPC):
                    simps.append(psp.tile([128, N], F32, name=f"simps{h}",
                                          tag="simps", bufs=2))
                for h in range(HPC):
                    for b in range(B):
                        nc.tensor.matmul(
                            simps[h][:, b * 512 + wlo:(b + 1) * 512],
                            kT[b][h * 64:(h + 1) * 64,
                                  jc * 128:(jc + 1) * 128],
                            qT[b][h * 64:(h + 1) * 64,
                                  ic * 512 + wlo:(ic + 1) * 512],
                            start=True, stop=False, skip_group_check=True)
                out = []
                for h in range(HPC):
                    for b in range(B):
                        # full-array bias add keeps the PE p-state high
                        nc.tensor.matmul(
                            simps[h][:, b * 512 + wlo:(b + 1) * 512],
                            identb[:],
                            ebt[(h, jc)][:, ic * 512 + wlo:(ic + 1) * 512],
                            start=False, stop=True, skip_group_check=True)
                    er = expp.tile([128, N], F16, name="expraw", tag="expraw")
                    if wlo == 0:
                        nc.scalar.activation(
                            er[:], simps[h][:],
                            mybir.ActivationFunctionType.Exp)
                    else:
                        nc.scalar.activation(
                            er[:].rearrange("p (b i) -> p b i",
                                            b=B)[:, :, wlo:512],
                            simps[h][:].rearrange("p (b i) -> p b i",
                                                  b=B)[:, :, wlo:512],
                            mybir.ActivationFunctionType.Exp)
                    out.append(er)
                return out

            def make_av():
                return [[psp.tile([DH + 1, 512], F32, name=f"av{hh}_{bb}",
                                  tag="avps", bufs=4) for bb in range(B)]
                        for hh in range(HPC)]

            def emit_av(av, h, jc, ic, e2, first, last):
                wlo = _wlo(ic, jc)
                for b in range(B):
                    nc.tensor.matmul(
                        av[h][b][:, wlo:512],
                        vaug_slice(b, h, jc),
                        e2[:, b * 512 + wlo:(b + 1) * 512],
                        start=first, stop=last,
                        skip_group_check=True)

            pend = []
            done_av = 0

            def flush_av(av, ic, jcs, upto):
                nonlocal done_av
                while done_av < upto:
                    jc, e2s = pend[done_av]
                    for h in range(HPC):
                        emit_av(av, h, jc, ic, e2s[h], done_av == 0,
                                done_av == len(jcs) - 1)
                    done_av += 1

            # ---- ic = 0 (first chunks fill the exp pipeline while
            #      the v projection runs on the PE) ----
            jcs0 = _unmasked_jcs(0)
            pend = []
            done_av = 0
            av0 = make_av()
            for jj, jc in enumerate(jcs0):
                pend.append((jc, emit_sims(0, jc, jj)))
                if jj == 1:
                    for b in range(B):
                        for icx in range(NIC):
                            v_proj_piece(b, icx)
                        v_transpose(b)
                if jj >= AV_DELAY:
                    flush_av(av0, 0, jcs0, jj - AV_DELAY + 1)
            flush_av(av0, 0, jcs0, len(jcs0))

            # ---- ic = 1 (ic0 normalize + out-proj interleaved) ----
            jcs1 = _unmasked_jcs(1)
            pend = []
            done_av = 0
            av1 = None
            for h in range(HPC):
                normalize(av0, 0, 0, h)
            for jj, jc in enumerate(jcs1):
                pend.append((jc, emit_sims(1, jc, jj)))
                if jj == 0:
                    av1 = make_av()
                    for h in range(HPC):
                        normalize(av0, 0, 1, h)
                if jj >= 3 and jj % 2 == 1:
                    i = (jj - 3) // 2
                    out_proj_half(i // 4, i % 4)
                if jj >= AV_DELAY:
                    flush_av(av1, 1, jcs1, jj - AV_DELAY + 1)
            flush_av(av1, 1, jcs1, len(jcs1))
            out_proj_half(1, 3)

            # ---- tail ----
            for b in range(B):
                for h in range(HPC):
                    normalize(av1, 1, b, h)
            for b in range(B):
                for ib in range(4, 8):
                    out_proj_half(b, ib)

    nc.compile()
    _NC_CACHE = nc
    return nc


def _prep_inputs(x, mem_k, mem_v, pos_bias, Wq, Wkv, Wo):
    """Build per-core input maps (host-side sharding)."""
    x = np.ascontiguousarray(x, dtype=np.float32)
    xT = np.ascontiguousarray(x.transpose(0, 2, 1)).astype(np.float16)

    import ml_dtypes
    pb = np.ascontiguousarray(
        pos_bias[0].transpose(0, 2, 1)).astype(np.float32)     # [16, JT, N]
    jj = np.arange(JT)[:, None]
    ii = np.arange(N)[None, :]
    mask = jj > (ii + M)
    eb = np.where(mask[None], np.float32(-240.0), pb).astype(
        ml_dtypes.float8_e4m3fn)

    def shuffle_w(w):  # [1024, 128] -> [128, 1024] with kc-major columns
        return np.ascontiguousarray(
            w.reshape(NKC, 128, 128).transpose(1, 0, 2).reshape(128, DIM))

    in_maps = []
    for c in range(NCORE):
        cs = 128 * c
        wq = shuffle_w(np.asarray(Wq[:, cs:cs + 128] * SCALE)).astype(np.float16)
        wk = shuffle_w(np.asarray(Wkv[:, cs:cs + 128])).astype(np.float16)
        wv = shuffle_w(
            np.asarray(Wkv[:, DIM + cs:DIM + cs + 128])).astype(np.float16)
        wo = np.ascontiguousarray(Wo[cs:cs + 128, :]).astype(np.float16)
        mkT = np.ascontiguousarray(
            mem_k[:, :, cs:cs + 128].transpose(0, 2, 1)).astype(np.float16)
        mv_s = mem_v[:, :, cs:cs + 128].astype(np.float16).reshape(
            B, NJ_MEM, 128, HPC, DH)
        mv = np.ones((B, HPC, 128, NJ_MEM, DH + 1), dtype=np.float16)
        for h in range(HPC):
            mv[:, h, :, :, 0:DH] = mv_s[:, :, :, h, :].transpose(0, 2, 1, 3)
        mv = mv.reshape(B, HPC, 128, NJ_MEM * (DH + 1))
        in_maps.append({
            "xT": xT,
            "wq": wq, "wk": wk, "wv": wv, "wo": wo,
            "mkT": mkT,
            "mv": np.ascontiguousarray(mv),
            "eb": np.ascontiguousarray(eb[2 * c:2 * c + 2]),
        })
    return in_maps


def kernel(x, mem_k, mem_v, pos_bias, Wq, Wkv, Wo, bo, **_kw):
    nc = _build()
    in_maps = _prep_inputs(
        np.asarray(x), np.asarray(mem_k), np.asarray(mem_v),
        np.asarray(pos_bias), np.asarray(Wq), np.asarray(Wkv), np.asarray(Wo))
    res = bass_utils.run_bass_kernel_spmd(nc, in_maps, core_ids=list(range(NCORE)))
    out = np.zeros((B, N, DIM), dtype=np.float64)
    for r in res.results:
        out += r["out"].astype(np.float64)
    out += np.asarray(bo, dtype=np.float64)[None, None, :]
    return out.astype(np.float32)

